# revision 1
# baseline (speedup 1.0000x reference)
"""Multi-head latent attention kernel for Trainium2, 8 NeuronCores.

Problem (hardcoded shapes):
  hidden_states [2, 2048, 4096] f32, attention_mask [1,1,2048,2048] f32,
  Wq [4096,4096], Wk/Wv [4096,1024], Wo [4096,4096].
  4 query heads x 1024 head_dim, 1 kv head, interleaved RoPE, softmax, o-proj.

Sharding: core c = (batch b=c//4, quarter r=c%4), all within-batch groups of 4.
  - k^T / v computed from the core's sequence quarter (hsq input) and
    AllGathered (one combined collective) within the 4-core batch group.
  - Attention is sharded over QUERY positions: each core handles global
    256-row i-blocks (r, 7-r) -- a causally balanced pairing -- for ALL 4
    heads, so the output projection is fully local and there is no second
    collective. The SPMD program uses the union block structure over the 4
    cores; per-core mask tiles make each core's softmax exact.
  - Scores are computed in S^T = k q^T orientation (partitions = key index)
    so exp(S^T) feeds the PV matmul directly; the softmax denominator comes
    from a ones-vector matmul and is applied via an outer-product broadcast
    of 1/l on PV evacuation.
All matmul operands bf16 (1 cycle/row on the PE vs 4 for f32), f32 PSUM
accumulation. RoPE is a host-side deinterleave permutation of Wq/Wk columns
plus 6 elementwise ops per (even,odd) chunk pair against cos/sin tables.
The mask is handled generically: each (local block, j-chunk) is classified
on host as skip / clean / mixed; mixed blocks add a (1/SCALE)-prescaled
per-core mask tile before the exp, so causal, zero, and arbitrary additive
masks are all supported (causal skips ~40% of attention compute).
"""

import numpy as np
import ml_dtypes

from concourse import bass, mybir, tile, bacc
from concourse import bass_utils

BF16 = mybir.dt.bfloat16
F32 = mybir.dt.float32

B, S, H = 2, 2048, 4096
NH, D = 4, 1024  # query heads, head dim
PD = D // 2  # rope pair count (512)
SCALE = D ** -0.5
NCORES = 8
GROUPS = [[0, 1, 2, 3], [4, 5, 6, 7]]

KC = H // 128  # 32 contraction chunks over hidden
DC = D // 128  # 8 d-chunks of head dim
SB = S // 512  # 4 s-blocks of 512
IB = S // 512  # 4 i-blocks (query) of 512
JC = S // 128  # 16 j-chunks (key) of 128
ICC = S // 128  # 16 i-chunks for o-proj
Q = S // 4  # 512, per-core kv sequence quarter

# results of the traced+profiled run (filled by kernel() when trace=True)
LAST_RESULTS = None


def _build(cats, n_mixed):
    """Build the SPMD bass program.

    cats: dict (lb, jc) -> "skip" | "clean" | int (index into packed mask
    tiles); lb in {0,1} is the local 256-row i-block, union over cores.
    """
    nc = bacc.Bacc("TRN2", target_bir_lowering=False, debug=False,
                   num_devices=NCORES)

    hsq_d = nc.dram_tensor("hsq", [H, Q], BF16, kind="ExternalInput")
    hsq2_d = nc.dram_tensor("hsq2", [H, 512], BF16, kind="ExternalInput")
    wq_d = nc.dram_tensor("wq", [H, H], BF16, kind="ExternalInput")
    wk_d = nc.dram_tensor("wk", [H, D], BF16, kind="ExternalInput")
    wv_d = nc.dram_tensor("wv", [H, D], BF16, kind="ExternalInput")
    wo_d = nc.dram_tensor("wo", [H, H], BF16, kind="ExternalInput")
    cosq_d = nc.dram_tensor("cosq", [PD, Q], BF16, kind="ExternalInput")
    sinq_d = nc.dram_tensor("sinq", [PD, Q], BF16, kind="ExternalInput")
    cosq2_d = nc.dram_tensor("cosq2", [PD, 512], BF16, kind="ExternalInput")
    sinq2_d = nc.dram_tensor("sinq2", [PD, 512], BF16, kind="ExternalInput")
    nmask = max(n_mixed, 1)
    maskp_d = nc.dram_tensor("maskp", [nmask * 128, 256], F32,
                             kind="ExternalInput")
    out_d = nc.dram_tensor("out", [512, H], F32, kind="ExternalOutput")

    # combined k+v collective bounce: rows [0:1024] = kT slice [1024, Q],
    # rows [1024:2048] = v slice [Q, 1024] flattened row-major to [1024, 512]
    kv_in = nc.dram_tensor("kv_in", [2048, 512], BF16, kind="Internal")
    kv_out = nc.dram_tensor("kv_out", [4 * 2048, 512], BF16, kind="Internal")

    PAIRS = [(0, 4), (1, 5), (2, 6), (3, 7)]

    with tile.TileContext(nc) as tc:
        with tc.tile_pool(name="pers", bufs=1) as pers:
            ones_col = pers.tile([128, 1], BF16, name="ones_col", tag="ones_col")
            nc.vector.memset(ones_col[:], 1.0)
            ones_row = pers.tile([1, 128], F32, name="ones_row", tag="ones_row")
            nc.vector.memset(ones_row[:], 1.0)
            # q^T for all 4 heads, local i columns: 32 chunks [128 d, 512 i]
            qT = [pers.tile([128, 512], BF16, name=f"qt{i}", tag=f"qt{i}")
                  for i in range(4 * DC)]

            # ============ phase A: projections + combined kv AG ============
            with (
                tc.tile_pool(name="pa", bufs=3) as pa,
                tc.tile_pool(name="paps", bufs=8, space="PSUM") as paps,
            ):
                hqc = [pa.tile([128, 8, Q], BF16, name=f"hqc{i}", tag=f"hqc{i}",
                               bufs=1) for i in range(4)]
                for i in range(4):
                    nc.sync.dma_start(
                        hqc[i][:],
                        hsq_d[1024 * i:1024 * (i + 1), :].rearrange(
                            "(kc p) s -> p kc s", p=128))

                def rope_pair(ps_e, ps_o, c_t, s_t, out_e, out_o, n):
                    """Stage psum pair to bf16, apply rope, write outputs."""
                    st_e = pa.tile([128, n], BF16, name="stg", tag="stg", bufs=6)
                    st_o = pa.tile([128, n], BF16, name="stg", tag="stg", bufs=6)
                    nc.scalar.activation(st_e[:], ps_e[:],
                                         mybir.ActivationFunctionType.Copy)
                    nc.scalar.activation(st_o[:], ps_o[:],
                                         mybir.ActivationFunctionType.Copy)
                    t1 = pa.tile([128, n], BF16, name="rtmp", tag="rtmp", bufs=4)
                    t2 = pa.tile([128, n], BF16, name="rtmp", tag="rtmp", bufs=4)
                    nc.vector.tensor_mul(t1[:], st_e[:], c_t)
                    nc.vector.tensor_mul(t2[:], st_o[:], s_t)
                    nc.vector.tensor_sub(out_e, t1[:], t2[:])
                    t3 = pa.tile([128, n], BF16, name="rtmp", tag="rtmp", bufs=4)
                    t4 = pa.tile([128, n], BF16, name="rtmp", tag="rtmp", bufs=4)
                    nc.vector.tensor_mul(t3[:], st_o[:], c_t)
                    nc.vector.tensor_mul(t4[:], st_e[:], s_t)
                    nc.vector.tensor_add(out_o, t3[:], t4[:])

                # --- K pass: kc-outer over 8 psum banks, then rope pairs ---
                kps = [paps.tile([128, Q], F32, name="mmps", tag="mmps")
                       for _ in range(DC)]
                for kc in range(KC):
                    wkt = pa.tile([128, D], BF16, name="wk", tag="wk", bufs=4)
                    nc.sync.dma_start(wkt[:], wk_d[128 * kc:128 * (kc + 1), :])
                    for dc in range(DC):
                        nc.tensor.matmul(
                            kps[dc][:], wkt[:, 128 * dc:128 * (dc + 1)],
                            hqc[kc // 8][:, kc % 8, :],
                            start=(kc == 0), stop=(kc == KC - 1))
                for pi, (de, do) in enumerate(PAIRS):
                    c_t = pa.tile([128, Q], BF16, name="ckt", tag="ckt", bufs=2)
                    s_t = pa.tile([128, Q], BF16, name="skt", tag="skt", bufs=2)
                    nc.sync.dma_start(c_t[:], cosq_d[128 * pi:128 * (pi + 1), :])
                    nc.sync.dma_start(s_t[:], sinq_d[128 * pi:128 * (pi + 1), :])
                    ke = pa.tile([128, Q], BF16, name="kout", tag="kout", bufs=4)
                    ko = pa.tile([128, Q], BF16, name="kout", tag="kout", bufs=4)
                    rope_pair(kps[de], kps[do], c_t[:], s_t[:], ke[:], ko[:], Q)
                    nc.sync.dma_start(kv_in[128 * de:128 * (de + 1), :], ke[:])
                    nc.sync.dma_start(kv_in[128 * do:128 * (do + 1), :], ko[:])

                # --- V pass: kc-outer over 8 psum banks ---
                vps = [paps.tile([128, 512], F32, name="mmps", tag="mmps")
                       for _ in range(8)]
                for kc in range(KC):
                    wvt = pa.tile([128, D], BF16, name="wv", tag="wv", bufs=4)
                    nc.sync.dma_start(wvt[:], wv_d[128 * kc:128 * (kc + 1), :])
                    for sc in range(4):
                        for dvb in range(2):
                            nc.tensor.matmul(
                                vps[sc * 2 + dvb][:],
                                hqc[kc // 8][:, kc % 8, 128 * sc:128 * (sc + 1)],
                                wvt[:, 512 * dvb:512 * (dvb + 1)],
                                start=(kc == 0), stop=(kc == KC - 1))
                for sc in range(4):
                    for dvb in range(2):
                        vt = pa.tile([128, 512], BF16, name="vout", tag="vout",
                                     bufs=4)
                        nc.scalar.activation(vt[:], vps[sc * 2 + dvb][:],
                                             mybir.ActivationFunctionType.Copy)
                        dst = kv_in[1024 + 256 * sc:1024 + 256 * (sc + 1), :]
                        dst = dst.rearrange("(p c) f -> p c f", c=2)[:, dvb, :]
                        nc.sync.dma_start(dst, vt[:])

                # --- combined kv AllGather (overlaps q pass) ---
                nc.gpsimd.collective_compute(
                    "AllGather", mybir.AluOpType.bypass, replica_groups=GROUPS,
                    ins=[kv_in.ap().opt()], outs=[kv_out.ap().opt()])

                # --- Q pass: all 4 heads x 8 d-chunks over local i cols ---
                hqc2 = [pa.tile([128, 8, 512], BF16, name=f"hqc2{i}",
                                tag=f"hqc2{i}", bufs=1) for i in range(4)]
                for i in range(4):
                    nc.sync.dma_start(
                        hqc2[i][:],
                        hsq2_d[1024 * i:1024 * (i + 1), :].rearrange(
                            "(kc p) s -> p kc s", p=128))
                for hp in range(4):  # process head h's 8 d-chunks as 4 pairs
                    qps = [paps.tile([128, 512], F32, name="mmps", tag="mmps")
                           for _ in range(DC)]
                    for kc in range(KC):
                        wqt = pa.tile([128, D], BF16, name="wqs", tag="wqs",
                                      bufs=12)
                        nc.sync.dma_start(
                            wqt[:], wq_d[128 * kc:128 * (kc + 1),
                                         D * hp:D * (hp + 1)])
                        for dc in range(DC):
                            nc.tensor.matmul(
                                qps[dc][:], wqt[:, 128 * dc:128 * (dc + 1)],
                                hqc2[kc // 8][:, kc % 8, :],
                                start=(kc == 0), stop=(kc == KC - 1))
                    for pi, (de, do) in enumerate(PAIRS):
                        c_t = pa.tile([128, 512], BF16, name="cqt", tag="cqt",
                                      bufs=3)
                        s_t = pa.tile([128, 512], BF16, name="sqt", tag="sqt",
                                      bufs=3)
                        nc.sync.dma_start(c_t[:],
                                          cosq2_d[128 * pi:128 * (pi + 1), :])
                        nc.sync.dma_start(s_t[:],
                                          sinq2_d[128 * pi:128 * (pi + 1), :])
                        rope_pair(qps[de], qps[do], c_t[:], s_t[:],
                                  qT[DC * hp + de][:], qT[DC * hp + do][:], 512)

            # ========== phase B: attention (all heads, local i-blocks) =====
            with tc.tile_pool(name="pb", bufs=2) as pb:
                kT = [pb.tile([128, S], BF16, name=f"kt{i}", tag=f"kt{i}",
                              bufs=1) for i in range(DC)]
                for dc in range(DC):
                    for r in range(4):
                        nc.sync.dma_start(
                            kT[dc][:, Q * r:Q * (r + 1)],
                            kv_out[2048 * r + 128 * dc:
                                   2048 * r + 128 * (dc + 1), :])
                vT = [pb.tile([128, D], BF16, name=f"vt{i}", tag=f"vt{i}",
                              bufs=1) for i in range(JC)]
                for jc in range(JC):
                    base = 2048 * (jc // 4) + 1024 + 256 * (jc % 4)
                    nc.sync.dma_start(
                        vT[jc][:],
                        kv_out[base:base + 256, :].rearrange(
                            "(p c) f -> p (c f)", c=2))
                # attention output, transposed: 32 chunks [128 hdv, 512 i]
                attnT = [pb.tile([128, 512], BF16, name=f"att{i}",
                                 tag=f"att{i}", bufs=1) for i in range(KC)]

                pbps_cm = tc.tile_pool(name="pbps", bufs=2, space="PSUM")
                pbps = pbps_cm.__enter__()
                for lb in range(2):
                    lsl = slice(256 * lb, 256 * (lb + 1))
                    live = [jc for jc in range(JC)
                            if cats[(lb, jc)] != "skip"]
                    for h in range(NH):
                        pT = {}
                        for jc in live:
                            sps = pbps.tile([128, 256], F32, name="sps",
                                            tag="sps", bufs=2)
                            for dc in range(DC):
                                nc.tensor.matmul(
                                    sps[:],
                                    kT[dc][:, 128 * jc:128 * (jc + 1)],
                                    qT[DC * h + dc][:, lsl],
                                    start=(dc == 0), stop=(dc == DC - 1))
                            cat = cats[(lb, jc)]
                            if isinstance(cat, int):
                                mt = pb.tile([128, 256], F32, name="mask",
                                             tag="mask", bufs=3)
                                nc.sync.dma_start(
                                    mt[:],
                                    maskp_d[128 * cat:128 * (cat + 1), :])
                                nc.vector.tensor_add(sps[:], sps[:], mt[:])
                            pt = pb.tile([128, 256], BF16, name=f"pt{jc}",
                                         tag=f"pt{jc}", bufs=2)
                            nc.scalar.activation(
                                pt[:], sps[:],
                                mybir.ActivationFunctionType.Exp, scale=SCALE)
                            pT[jc] = pt
                        l_ps = pbps.tile([1, 256], F32, name="lps", tag="lps",
                                         bufs=1)
                        for n, jc in enumerate(live):
                            nc.tensor.matmul(l_ps[:], ones_col[:], pT[jc][:],
                                             start=(n == 0),
                                             stop=(n == len(live) - 1))
                        r_sb = pb.tile([1, 256], F32, name="rsb", tag="rsb",
                                       bufs=2)
                        nc.vector.reciprocal(r_sb[:], l_ps[:])
                        r_ps = pbps.tile([128, 256], F32, name="rps",
                                         tag="rps", bufs=1)
                        nc.tensor.matmul(r_ps[:], ones_row[:], r_sb[:],
                                         start=True, stop=True)
                        rbc = pb.tile([128, 256], F32, name="rbc", tag="rbc",
                                      bufs=2)
                        nc.scalar.activation(rbc[:], r_ps[:],
                                             mybir.ActivationFunctionType.Copy)
                        for dc2 in range(DC):
                            pvps = pbps.tile([128, 256], F32, name="pvps",
                                             tag="pvps", bufs=2)
                            for n, jc in enumerate(live):
                                nc.tensor.matmul(
                                    pvps[:],
                                    vT[jc][:, 128 * dc2:128 * (dc2 + 1)],
                                    pT[jc][:], start=(n == 0),
                                    stop=(n == len(live) - 1))
                            nc.vector.tensor_mul(
                                attnT[DC * h + dc2][:, lsl], pvps[:], rbc[:])

                pbps_cm.__exit__(None, None, None)

                # ============ phase C: local output projection ============
                with (
                    tc.tile_pool(name="pc", bufs=2) as pc,
                    tc.tile_pool(name="pcps", bufs=4, space="PSUM") as pcps,
                ):
                    for eb in range(8):
                        ops = [pcps.tile([128, 512], F32, name="ops",
                                         tag="ops") for _ in range(4)]
                        for kc in range(KC):
                            wot = pc.tile([128, 512], BF16, name="wot",
                                          tag="wot", bufs=8)
                            nc.sync.dma_start(
                                wot[:], wo_d[128 * kc:128 * (kc + 1),
                                             512 * eb:512 * (eb + 1)])
                            for ic in range(4):
                                nc.tensor.matmul(
                                    ops[ic][:],
                                    attnT[kc][:, 128 * ic:128 * (ic + 1)],
                                    wot[:], start=(kc == 0),
                                    stop=(kc == KC - 1))
                        for ic in range(4):
                            ot = pc.tile([128, 512], F32, name="otile",
                                         tag="otile", bufs=4)
                            nc.vector.tensor_copy(ot[:], ops[ic][:])
                            nc.sync.dma_start(
                                out_d[128 * ic:128 * (ic + 1),
                                      512 * eb:512 * (eb + 1)], ot[:])

    nc.compile()
    return nc


_BUILD_CACHE = {}

# core r (within its batch group) handles global 256-row i-blocks (r, 7-r)
GMAP = [(r, 7 - r) for r in range(4)]


def _classify_mask(mask):
    """Union-classify each (local block lb, jc) over the 4 quarter cores.

    Returns (cats, per-core packed mask tile arrays, n_mixed). The program
    structure (cats) is shared by all cores; mask tiles are per-core data.
    """
    m = np.asarray(mask).reshape(S, S)  # [i, j]
    cats = {}
    tiles = [[] for _ in range(4)]
    n = 0
    for lb in range(2):
        for jc in range(JC):
            blks = [m[256 * GMAP[r][lb]:256 * (GMAP[r][lb] + 1),
                      128 * jc:128 * (jc + 1)] for r in range(4)]
            if all(np.all(b <= -1e8) for b in blks):
                cats[(lb, jc)] = "skip"
            elif not any(b.any() for b in blks):
                cats[(lb, jc)] = "clean"
            else:
                cats[(lb, jc)] = n
                n += 1
                for r in range(4):
                    # [j, i] orientation, prescaled by 1/SCALE so the ACT's
                    # uniform SCALE reproduces reference's scores*SCALE + mask
                    tiles[r].append(
                        np.ascontiguousarray(blks[r].T) * (1.0 / SCALE))
    maskps = [
        np.concatenate(t, axis=0).astype(np.float32) if t
        else np.zeros((128, 256), np.float32) for t in tiles]
    return cats, maskps, n


def kernel(hidden_states, attention_mask, Wq, Wk, Wv, Wo, trace=False):
    global LAST_RESULTS
    bf = ml_dtypes.bfloat16

    cats, maskps, n_mixed = _classify_mask(attention_mask)
    key = tuple(sorted((k, v if isinstance(v, str) else "m")
                       for k, v in cats.items()))
    if key not in _BUILD_CACHE:
        _BUILD_CACHE[key] = _build(cats, n_mixed)
    nc = _BUILD_CACHE[key]

    # deinterleave rope pairs within each head's 1024 columns
    perm = np.concatenate([np.arange(0, D, 2), np.arange(1, D, 2)])
    cols = np.concatenate([h * D + perm for h in range(NH)])
    wq_p = np.ascontiguousarray(Wq[:, cols]).astype(bf)
    wk_p = np.ascontiguousarray(Wk[:, perm]).astype(bf)
    wv_c = np.ascontiguousarray(Wv).astype(bf)
    wo_c = np.ascontiguousarray(Wo).astype(bf)

    freqs = 1.0 / (10000.0 ** (np.arange(0, D, 2, dtype=np.float64) / D))
    ang = np.outer(np.arange(S, dtype=np.float64), freqs)  # [S, PD]
    cosT = np.ascontiguousarray(np.cos(ang).T).astype(bf)  # [PD, S]
    sinT = np.ascontiguousarray(np.sin(ang).T).astype(bf)

    hsT = [np.ascontiguousarray(hidden_states[b].T).astype(bf)
           for b in range(B)]

    in_maps = []
    for c in range(NCORES):
        b, r = c // 4, c % 4
        g0, g1 = GMAP[r]
        icols = np.r_[256 * g0:256 * (g0 + 1), 256 * g1:256 * (g1 + 1)]
        in_maps.append({
            "hsq": np.ascontiguousarray(hsT[b][:, Q * r:Q * (r + 1)]),
            "hsq2": np.ascontiguousarray(hsT[b][:, icols]),
            "wq": wq_p,
            "wk": wk_p,
            "wv": wv_c,
            "wo": wo_c,
            "cosq": np.ascontiguousarray(cosT[:, Q * r:Q * (r + 1)]),
            "sinq": np.ascontiguousarray(sinT[:, Q * r:Q * (r + 1)]),
            "cosq2": np.ascontiguousarray(cosT[:, icols]),
            "sinq2": np.ascontiguousarray(sinT[:, icols]),
            "maskp": maskps[r],
        })

    res = bass_utils.run_bass_kernel_spmd(
        nc, in_maps, core_ids=list(range(NCORES)), trace=trace)
    LAST_RESULTS = res

    out = np.empty((B, S, H), np.float32)
    for c in range(NCORES):
        b, r = c // 4, c % 4
        g0, g1 = GMAP[r]
        o = res.results[c]["out"]
        out[b, 256 * g0:256 * (g0 + 1), :] = o[0:256]
        out[b, 256 * g1:256 * (g1 + 1), :] = o[256:512]
    return out

